# revision 13
# baseline (speedup 1.0000x reference)
"""Trainium2 Bass kernel for nn_MoE_81209241633272.

MoE layer: 16 experts, top-4 routing, gated-SiLU expert MLPs (2048->1024->2048)
plus an always-on shared gated MLP (2048->2048->2048), over 4096 tokens.

Strategy (expert-parallel across 8 cores):
  - Each core computes 2 experts (dense over all tokens; router coef zeroes the
    non-selected tokens) plus a 1/8 column-slice of the shared MLP.
  - Activations live in transposed layout x^T [D, T] so every matmul uses the
    weights in their natural layout and chains without transposes:
        h^T[I,T]  = matmul(lhsT=w1[D,I] tiles, rhs=x^T)         (PSUM [128,512])
        y [T,D]   = matmul(lhsT=h^T tiles,     rhs=w2[I,D])     (PSUM [128,512])
  - Router: logits computed bit-accurately via split bf16 (hi/lo) operands:
        logits = gh@xh + (gl@xh + gh@xl)   (lo*lo term negligible)
    done as two accumulation groups into one PSUM [32, 512] using packed gate
    matrices, then PE-transpose to [T,16] for softmax + top-4 thresholding.
    Per-core gate columns are permuted so that this core's experts are always
    columns 0 and 1 (softmax/top-k are permutation invariant); the top-4 mask
    is computed on raw logits so expert selection is bit-identical across cores.
  - Per chunk of 512 tokens the partial y (2 experts + shared slice) is
    ReduceScatter'd (sum) across the 8 cores; host concatenates the slices.

All matmuls are bf16 inputs with fp32 PSUM accumulation; everything else
(softmax, coef, y accumulation) is fp32.
"""

import numpy as np
import ml_dtypes

import concourse.bass as bass
import concourse.bacc as bacc
import concourse.mybir as mybir
from concourse.tile import TileContext

BF16 = ml_dtypes.bfloat16
F32 = np.float32

N_CORES = 8
P = 128
B, S = 4, 1024
T = B * S              # 4096 tokens
D = 2048               # model dim
E = 16                 # experts
TOP_K = 4
I_EXP = 1024           # expert inter dim
SH_INTER = 2048        # shared inter dim (total)
SH_PC = SH_INTER // N_CORES  # shared inter slice per core = 256

CH = 512               # tokens per chunk (PSUM free-dim limit for fp32)
KO = D // P            # 16 k-tiles over D
IEO = I_EXP // P       # 8 i-tiles per expert
ISO = SH_PC // P       # 2 i-tiles for shared slice
DCH = 512              # output D chunk
NDCH = D // DCH        # 4
NTS = CH // P          # 4 token-slices per chunk
RS_OUT = CH // N_CORES  # 64 rows per core from each chunk's reduce-scatter

AX = mybir.AxisListType
ALU = mybir.AluOpType
ACT = mybir.ActivationFunctionType
dt = mybir.dt


def build_nc(n_chunks=T // CH):
    nc = bacc.Bacc("TRN2", target_bir_lowering=False, num_devices=N_CORES)

    # ---- kernel I/O (per-core tensors; host supplies core-specific data) ----
    xh_d = nc.dram_tensor("xh", [n_chunks, P, KO, CH], dt.bfloat16, kind="ExternalInput")
    xl_d = nc.dram_tensor("xl", [n_chunks, P, KO, CH], dt.bfloat16, kind="ExternalInput")
    w1a_d = nc.dram_tensor("w1a", [P, KO, I_EXP], dt.bfloat16, kind="ExternalInput")
    w3a_d = nc.dram_tensor("w3a", [P, KO, I_EXP], dt.bfloat16, kind="ExternalInput")
    w2a_d = nc.dram_tensor("w2a", [P, IEO, D], dt.bfloat16, kind="ExternalInput")
    w1b_d = nc.dram_tensor("w1b", [P, KO, I_EXP], dt.bfloat16, kind="ExternalInput")
    w3b_d = nc.dram_tensor("w3b", [P, KO, I_EXP], dt.bfloat16, kind="ExternalInput")
    w2b_d = nc.dram_tensor("w2b", [P, IEO, D], dt.bfloat16, kind="ExternalInput")
    ws13_d = nc.dram_tensor("ws13", [P, KO, 2 * SH_PC], dt.bfloat16, kind="ExternalInput")
    ws2_d = nc.dram_tensor("ws2", [P, ISO, D], dt.bfloat16, kind="ExternalInput")
    g1_d = nc.dram_tensor("g1", [P, KO, 3 * E], dt.bfloat16, kind="ExternalInput")
    g2_d = nc.dram_tensor("g2", [P, KO, 3 * E], dt.bfloat16, kind="ExternalInput")

    y_out = nc.dram_tensor("y_out", [n_chunks, RS_OUT, D], dt.float32, kind="ExternalOutput")

    # internal DRAM for the collective (collectives can't touch kernel I/O)
    y_part = nc.dram_tensor("y_part", [n_chunks, CH, D], dt.float32)
    y_rs = nc.dram_tensor("y_rs", [n_chunks, RS_OUT, D], dt.float32)

    with TileContext(nc) as tc:
        with (
            tc.tile_pool(name="const", bufs=1) as cpool,
            tc.tile_pool(name="xp", bufs=1) as xpool,
            tc.tile_pool(name="wp", bufs=3) as wpool,
            tc.tile_pool(name="hp", bufs=1) as hpool,
            tc.tile_pool(name="hsp", bufs=2) as hspool,
            tc.tile_pool(name="sp", bufs=3) as spool,
            tc.tile_pool(name="yp", bufs=3) as ypool,
            tc.tile_pool(name="gp", bufs=2) as gpool,
            tc.tile_pool(name="smp", bufs=2) as smpool,
            tc.tile_pool(name="php", bufs=3, space="PSUM") as php,
            tc.tile_pool(name="pgp", bufs=2, space="PSUM") as pgp,
            tc.tile_pool(name="pyp", bufs=3, space="PSUM") as pyp,
        ):
            # ---- resident constants ----
            # identity re-emitted by the DVE so the transpose matmuls depend
            # on a single semaphore (LDW weight-loads only fit one sync wait)
            ident_g = cpool.tile([E, E], dt.float32, tag="ident_g")
            from concourse.masks import make_identity
            make_identity(nc, ident_g)
            ident = cpool.tile([E, E], dt.float32, tag="ident")
            nc.vector.tensor_copy(ident, ident_g)
            g1_sb = cpool.tile([P, KO, 3 * E], dt.bfloat16, tag="g1")
            nc.sync.dma_start(g1_sb, g1_d[:])
            g2_sb = cpool.tile([P, KO, 3 * E], dt.bfloat16, tag="g2")
            nc.sync.dma_start(g2_sb, g2_d[:])
            ws13_sb = cpool.tile([P, KO, 2 * SH_PC], dt.bfloat16, tag="ws13")
            nc.sync.dma_start(ws13_sb, ws13_d[:])
            ws2_sb = cpool.tile([P, ISO, D], dt.bfloat16, tag="ws2")
            nc.sync.dma_start(ws2_sb, ws2_d[:])

            for c in range(n_chunks):
                # ---- stream this chunk's activations ----
                xh_sb = xpool.tile([P, KO, CH], dt.bfloat16, tag="xh")
                nc.sync.dma_start(xh_sb, xh_d[c])
                xl_sb = xpool.tile([P, KO, CH], dt.bfloat16, tag="xl")
                nc.sync.dma_start(xl_sb, xl_d[c])

                # ---- gate: logits^T [16, CH] via packed split-precision matmuls ----
                # pg rows 0:16 = gh@xh ; rows 32:48 = gl@xh + gh@xl (32-aligned base)
                pg = pgp.tile([48, CH], dt.float32, tag="pg")
                for ko in range(KO):
                    nc.tensor.matmul(pg, g1_sb[:, ko, :], xh_sb[:, ko, :],
                                     start=(ko == 0), stop=False)
                for ko in range(KO):
                    nc.tensor.matmul(pg, g2_sb[:, ko, :], xl_sb[:, ko, :],
                                     start=False, stop=(ko == KO - 1))
                logits_hi = gpool.tile([16, CH], dt.float32, tag="lgh")
                nc.vector.tensor_copy(logits_hi, pg[0:16, :])
                logits_sb = gpool.tile([16, CH], dt.float32, tag="lg")
                nc.vector.tensor_add(logits_sb, logits_hi, pg[32:48, :])

                # ---- softmax + exact top-4 per token-slice ----
                coef_c = gpool.tile([P, NTS, E], dt.float32, tag="coef")
                for t in range(NTS):
                    # transpose [16,128] -> [128,16] as a regular tiny matmul:
                    # out[m,n] = sum_k logits[k,m] * I16[k,n] = logits[n,m]
                    pt = pgp.tile([P, E], dt.float32, tag="pg")
                    nc.tensor.matmul(pt, logits_sb[:, t * P:(t + 1) * P],
                                     ident, start=True, stop=True)
                    # softmax (max-subtracted, fp32)
                    mx = smpool.tile([P, 1], dt.float32, tag="mx")
                    nc.vector.reduce_max(mx, pt, axis=AX.X)
                    nm = smpool.tile([P, 1], dt.float32, tag="nm")
                    nc.vector.tensor_scalar_mul(nm, mx, -1.0)
                    ex = smpool.tile([P, E], dt.float32, tag="ex")
                    ssum = smpool.tile([P, 1], dt.float32, tag="ss")
                    nc.scalar.activation(ex, pt, ACT.Exp, bias=nm, scale=1.0,
                                         accum_out=ssum)
                    rcp = smpool.tile([P, 1], dt.float32, tag="rc")
                    nc.vector.reciprocal(rcp, ssum)
                    probs = smpool.tile([P, E], dt.float32, tag="pr")
                    nc.vector.tensor_scalar_mul(probs, ex, rcp)
                    # 4th-largest logit as threshold (bit-identical across cores)
                    work = smpool.tile([P, E], dt.float32, tag="wk")
                    nc.vector.tensor_copy(work, pt)
                    for _ in range(TOP_K - 1):
                        m = smpool.tile([P, 1], dt.float32, tag="m")
                        nc.vector.reduce_max(m, work, axis=AX.X)
                        msk = smpool.tile([P, E], dt.float32, tag="msk")
                        nc.vector.tensor_scalar(msk, work, m, 1.0e4,
                                                op0=ALU.is_ge, op1=ALU.mult)
                        nc.vector.tensor_sub(work, work, msk)
                    m4 = smpool.tile([P, 1], dt.float32, tag="m4")
                    nc.vector.reduce_max(m4, work, axis=AX.X)
                    gem = smpool.tile([P, E], dt.float32, tag="gem")
                    nc.vector.tensor_scalar(gem, pt, m4, None, op0=ALU.is_ge)
                    nc.vector.tensor_mul(coef_c[:, t, :], probs, gem)

                # ---- shared-expert h (resident weights) ----
                hs = hspool.tile([P, ISO, CH], dt.bfloat16, tag="hs")
                for i in range(ISO):
                    p1 = php.tile([P, CH], dt.float32, tag="ph")
                    for ko in range(KO):
                        nc.tensor.matmul(p1, ws13_sb[:, ko, i * P:(i + 1) * P],
                                         xh_sb[:, ko, :],
                                         start=(ko == 0), stop=(ko == KO - 1))
                    p3 = php.tile([P, CH], dt.float32, tag="ph")
                    for ko in range(KO):
                        nc.tensor.matmul(p3, ws13_sb[:, ko, SH_PC + i * P:SH_PC + (i + 1) * P],
                                         xh_sb[:, ko, :],
                                         start=(ko == 0), stop=(ko == KO - 1))
                    sl = spool.tile([P, CH], dt.bfloat16, tag="sl")
                    nc.scalar.activation(sl, p1, ACT.Sigmoid)
                    nc.vector.tensor_mul(sl, sl, p1)
                    nc.vector.tensor_mul(hs[:, i, :], sl, p3)

                # ---- expert h phases (streamed weights) ----
                # load order cycles the 3 weight slots for max prefetch overlap
                w1a_sb = wpool.tile([P, KO, I_EXP], dt.bfloat16, tag="w")
                nc.sync.dma_start(w1a_sb, w1a_d[:])
                w3a_sb = wpool.tile([P, KO, I_EXP], dt.bfloat16, tag="w")
                nc.sync.dma_start(w3a_sb, w3a_d[:])
                w1b_sb = wpool.tile([P, KO, I_EXP], dt.bfloat16, tag="w")
                nc.sync.dma_start(w1b_sb, w1b_d[:])

                h_tiles = []
                w2a_sb = None
                for ei in range(2):
                    if ei == 0:
                        w1_sb, w3_sb = w1a_sb, w3a_sb
                    else:
                        # second expert's w3 loads into the slot freed by w1a
                        w3b_sb = wpool.tile([P, KO, I_EXP], dt.bfloat16, tag="w")
                        nc.sync.dma_start(w3b_sb, w3b_d[:])
                        w1_sb, w3_sb = w1b_sb, w3b_sb
                    he = hpool.tile([P, IEO, CH], dt.bfloat16, tag=f"h{ei}")
                    for i in range(IEO):
                        p1 = php.tile([P, CH], dt.float32, tag="ph")
                        for ko in range(KO):
                            nc.tensor.matmul(p1, w1_sb[:, ko, i * P:(i + 1) * P],
                                             xh_sb[:, ko, :],
                                             start=(ko == 0), stop=(ko == KO - 1))
                        p3 = php.tile([P, CH], dt.float32, tag="ph")
                        for ko in range(KO):
                            nc.tensor.matmul(p3, w3_sb[:, ko, i * P:(i + 1) * P],
                                             xh_sb[:, ko, :],
                                             start=(ko == 0), stop=(ko == KO - 1))
                        sl = spool.tile([P, CH], dt.bfloat16, tag="sl")
                        nc.scalar.activation(sl, p1, ACT.Sigmoid)
                        nc.vector.tensor_mul(sl, sl, p1)
                        nc.vector.tensor_mul(he[:, i, :], sl, p3)
                    h_tiles.append(he)
                    if ei == 0:
                        w2a_sb = wpool.tile([P, IEO, D], dt.bfloat16, tag="w")
                        nc.sync.dma_start(w2a_sb, w2a_d[:])
                h0, h1 = h_tiles
                w2b_sb = wpool.tile([P, IEO, D], dt.bfloat16, tag="w")
                nc.sync.dma_start(w2b_sb, w2b_d[:])

                # ---- phase 2: y[T,D] per (t, d) tile; combine with router coef ----
                for t in range(NTS):
                    tsl = slice(t * P, (t + 1) * P)
                    for d in range(NDCH):
                        dsl = slice(d * DCH, (d + 1) * DCH)
                        py0 = pyp.tile([P, DCH], dt.float32, tag="py")
                        for i in range(IEO):
                            nc.tensor.matmul(py0, h0[:, i, tsl], w2a_sb[:, i, dsl],
                                             start=(i == 0), stop=(i == IEO - 1))
                        py1 = pyp.tile([P, DCH], dt.float32, tag="py")
                        for i in range(IEO):
                            nc.tensor.matmul(py1, h1[:, i, tsl], w2b_sb[:, i, dsl],
                                             start=(i == 0), stop=(i == IEO - 1))
                        pys = pyp.tile([P, DCH], dt.float32, tag="py")
                        for i in range(ISO):
                            nc.tensor.matmul(pys, hs[:, i, tsl], ws2_sb[:, i, dsl],
                                             start=(i == 0), stop=(i == ISO - 1))
                        y_t = ypool.tile([P, DCH], dt.float32, tag="yt")
                        nc.vector.tensor_scalar_mul(y_t, py0, coef_c[:, t, 0:1])
                        nc.vector.scalar_tensor_tensor(y_t, py1, coef_c[:, t, 1:2],
                                                       y_t, op0=ALU.mult, op1=ALU.add)
                        nc.vector.tensor_add(y_t, y_t, pys)
                        nc.sync.dma_start(y_part[c, tsl, dsl], y_t)

                # ---- reduce across cores; each core keeps its 64-row slice ----
                nc.gpsimd.collective_compute(
                    "ReduceScatter",
                    ALU.add,
                    replica_groups=[list(range(N_CORES))],
                    ins=[y_part[c].opt()],
                    outs=[y_rs[c].opt()],
                )
                nc.sync.dma_start(y_out[c], y_rs[c])

    nc.finalize()
    return nc


# ---------------- host-side data prep ----------------

def _x_layout(a, n_chunks):
    # [T, D] -> [n_chunks, P(ki), KO, CH]  (partition line = KO*CH contiguous)
    t_use = n_chunks * CH
    return np.ascontiguousarray(
        a[:t_use].reshape(n_chunks, CH, KO, P).transpose(0, 3, 2, 1))


def _lhs_layout(w):
    # [D, N] -> [P(ki), D//P(ko), N]
    d, n = w.shape
    return np.ascontiguousarray(w.reshape(d // P, P, n).transpose(1, 0, 2))


def _hilo(a):
    hi = a.astype(BF16)
    lo = (a - hi.astype(F32)).astype(BF16)
    return hi, lo


def make_in_maps(inputs, n_chunks=T // CH):
    x = np.asarray(inputs["x"], F32).reshape(T, D)
    gate_w = np.asarray(inputs["gate_w"], F32)
    w1 = np.asarray(inputs["w1"], F32)
    w2 = np.asarray(inputs["w2"], F32)
    w3 = np.asarray(inputs["w3"], F32)
    ws1 = np.asarray(inputs["ws1"], F32)
    ws2 = np.asarray(inputs["ws2"], F32)
    ws3 = np.asarray(inputs["ws3"], F32)

    xh, xl = _hilo(x)
    xh_t = _x_layout(xh, n_chunks)
    xl_t = _x_layout(xl, n_chunks)

    in_maps = []
    for core in range(N_CORES):
        ea, eb = 2 * core, 2 * core + 1
        cols = slice(core * SH_PC, (core + 1) * SH_PC)
        ws13 = np.concatenate([ws1[:, cols], ws3[:, cols]], axis=1)

        perm = [ea, eb] + [e for e in range(E) if e not in (ea, eb)]
        gp = gate_w[:, perm]
        gh, gl = _hilo(gp)
        z = np.zeros_like(gh)
        g1 = np.concatenate([gh, z, gl], axis=1)
        g2 = np.concatenate([z, z, gh], axis=1)

        in_maps.append({
            "xh": xh_t, "xl": xl_t,
            "w1a": _lhs_layout(w1[ea].astype(BF16)),
            "w3a": _lhs_layout(w3[ea].astype(BF16)),
            "w2a": _lhs_layout(w2[ea].astype(BF16)),
            "w1b": _lhs_layout(w1[eb].astype(BF16)),
            "w3b": _lhs_layout(w3[eb].astype(BF16)),
            "w2b": _lhs_layout(w2[eb].astype(BF16)),
            "ws13": _lhs_layout(ws13.astype(BF16)),
            "ws2": _lhs_layout(ws2[cols].astype(BF16)),
            "g1": _lhs_layout(g1),
            "g2": _lhs_layout(g2),
        })
    return in_maps


def assemble_output(results, n_chunks=T // CH):
    y = np.zeros((n_chunks * CH, D), F32)
    for core in range(N_CORES):
        r = np.asarray(results[core]["y_out"])
        for c in range(n_chunks):
            y[c * CH + core * RS_OUT:(c * CH + (core + 1) * RS_OUT)] = r[c]
    return y


_NC_CACHE = {}


def kernel(**inputs) -> np.ndarray:
    from concourse.bass_utils import run_bass_kernel_spmd

    n_chunks = T // CH
    if n_chunks not in _NC_CACHE:
        _NC_CACHE[n_chunks] = build_nc(n_chunks)
    nc = _NC_CACHE[n_chunks]

    in_maps = make_in_maps(inputs, n_chunks)
    res = run_bass_kernel_spmd(nc, in_maps, core_ids=list(range(N_CORES)))
    y = assemble_output(res.results, n_chunks)
    return y.reshape(B, S, D)
